# revision 1
# baseline (speedup 1.0000x reference)
"""CrossAttention kernel for 8 trn2 NeuronCores.

Sharding: core = (batch b in 0..3, key-half h in 0..1).
Each core computes, for its batch b and its half of the keys:
    qT   = (Wq @ query[b].T + bq)      [E=1024, Sq=2048]
           (computed as a half per core, pair-AllGathered when USE_COLLECTIVE)
    kT   = (Wk @ key_half.T + bk)      [E=1024, Skv=1024]
    v    = (value_half @ Wv.T)         [Skv=1024, E=1024]  (bias deferred to host)
    sT   = scoresT[j,i] = k_j . q_i    [Skv, Sq]
    eT   = exp(sT / sqrt(D))           (no max subtraction; scores are O(1))
    outT = outT[d,i] = sum_j v[j,d] eT[j,i]   [E, Sq]  (unnormalized)
    sums = sum_j eT[j,i]               [1, Sq]  (ones-matmul)
Host combines the two halves per batch:
    out[b] = ((outT0+outT1) / (sums0+sums1)).T + bv
All matmuls run in bf16 with fp32 PSUM accumulation.
"""

from contextlib import ExitStack

import numpy as np
import ml_dtypes

import concourse.bass as bass
import concourse.tile as tile
from concourse import bacc, mybir
from concourse.bass_utils import run_bass_kernel_spmd

BF16 = mybir.dt.bfloat16
FP32 = mybir.dt.float32

B = 4
SQ = 2048        # query length (full batch)
SQH = SQ // 2    # per-core query half (collective mode)
SKV = 1024       # keys per core (half of 2048)
D = 1024         # model dim = proj dim
P = 128          # partitions
CH = 512         # psum free-dim chunk
DT = D // P      # 8 contraction tiles for projections
ET = D // P      # 8 e-tiles
JT = SKV // P    # 8 key tiles per core
NCH = SQ // CH   # 4 sq chunks
SCALE = 1.0 / float(np.sqrt(D))

USE_COLLECTIVE = True

LAST_EXEC_NS = None
LAST_RESULT = None


def _split_multi_waits(nc):
    """The container's walrus supports exactly ONE sync-wait command per
    instruction ("Too many sync wait commands" otherwise). Tile emits
    instructions carrying several waits; split the extras onto same-engine
    NOPs inserted immediately before the instruction (engine streams are
    in-order, so waits still complete before the instruction starts)."""
    ctr = 0
    for fn in nc.m.functions:
        for bb in fn.blocks:
            insts = bb.instructions
            new = []
            changed = False
            for inst in insts:
                si = inst.sync_info
                waits = list(si.on_wait) if si is not None and si.on_wait else []
                if len(waits) > 1:
                    changed = True
                    for w in waits[:-1]:
                        ctr += 1
                        new.append(
                            mybir.InstNoOp(
                                name=f"waitsplit_{ctr}",
                                engine=inst.engine,
                                ins=[],
                                outs=[],
                                sync_info=mybir.SyncInfo(on_wait=[w], on_update=[]),
                            )
                        )
                    inst.sync_info = mybir.SyncInfo(
                        on_wait=[waits[-1]],
                        on_update=list(si.on_update) if si.on_update else [],
                    )
                new.append(inst)
            if changed:
                insts[:] = new
    return ctr


class _SlimTailTileContext(tile.TileContext):
    """Tile's kernel tail is drain + all-engine barrier + semaphore
    range-clear + second barrier (~10 us on HW). Only the drain (with its
    global-clock waits) is needed for the outputs of THIS execution to be
    complete when every engine halts; the clears/barriers are hygiene for
    re-executing the same loaded NEFF, which we never do."""

    def _drain_and_barrier(self, tick_clock, wait_clock):
        from concourse.vector_clock import ScopedClock

        drain_inst = self.nc.sync.drain()
        wait_clock.add_sem_waits(
            drain_inst.ins, ScopedClock({None: tick_clock.global_clock})
        )
        assert self.sems is not None
        popped = self.nc._tile_sem_poison_stack.pop()
        assert popped is self._sem_poison


def _build_bass():
    nc = bacc.Bacc(
        "TRN2", target_bir_lowering=False, debug=False, num_devices=8
    )

    sq_in = SQH if USE_COLLECTIVE else SQ
    xqT_d = nc.dram_tensor("xqT", [D, sq_in], BF16, kind="ExternalInput")
    xkT_d = nc.dram_tensor("xkT", [D, SKV], BF16, kind="ExternalInput")
    xvT_d = nc.dram_tensor("xvT", [D, SKV], BF16, kind="ExternalInput")
    wqT_d = nc.dram_tensor("wqT", [D, D], BF16, kind="ExternalInput")
    wkT_d = nc.dram_tensor("wkT", [D, D], BF16, kind="ExternalInput")
    wvT_d = nc.dram_tensor("wvT", [D, D], BF16, kind="ExternalInput")
    bqr_d = nc.dram_tensor("bqr", [P, DT], FP32, kind="ExternalInput")
    bkr_d = nc.dram_tensor("bkr", [P, DT], FP32, kind="ExternalInput")
    outT_d = nc.dram_tensor("outT", [D, SQ], FP32, kind="ExternalOutput")
    sums_d = nc.dram_tensor("sums", [1, SQ], FP32, kind="ExternalOutput")

    with _SlimTailTileContext(nc) as tc, ExitStack() as ctx:
        const_pool = ctx.enter_context(tc.tile_pool(name="const", bufs=1))
        persist = ctx.enter_context(tc.tile_pool(name="persist", bufs=1))
        # attention pools allocated BEFORE the wx scope so their SBUF space
        # does not overlap wx's — otherwise the first exp tiles wait for the
        # last projection reads before they can allocate
        exp_pool = ctx.enter_context(tc.tile_pool(name="expp", bufs=12))
        stage = ctx.enter_context(tc.tile_pool(name="stage", bufs=4))
        psum_proj = ctx.enter_context(
            tc.tile_pool(name="psum_proj", bufs=3, space="PSUM")
        )
        psum_s = ctx.enter_context(tc.tile_pool(name="psum_s", bufs=2, space="PSUM"))
        psum_o = ctx.enter_context(tc.tile_pool(name="psum_o", bufs=2, space="PSUM"))
        psum_n = ctx.enter_context(tc.tile_pool(name="psum_n", bufs=1, space="PSUM"))

        ones_sb = const_pool.tile([P, 1], BF16)
        nc.vector.memset(ones_sb, 1.0)
        bq_sb = const_pool.tile([P, DT], FP32)
        nc.sync.dma_start(out=bq_sb, in_=bqr_d[:, :])
        bk_sb = const_pool.tile([P, DT], FP32)
        nc.sync.dma_start(out=bk_sb, in_=bkr_d[:, :])

        # persistent projection outputs (bf16)
        qT_sb = persist.tile([P, ET, SQ], BF16)   # [e_in, e_out, sq]
        kT_sb = persist.tile([P, ET, SKV], BF16)  # [e_in, e_out, skv]
        v_sb = persist.tile([P, JT, D], BF16)     # [j_in, j_out, e]

        if USE_COLLECTIVE:
            dram = ctx.enter_context(tc.tile_pool(name="dram", bufs=1, space="DRAM"))
            qTh_dram = dram.tile([D, SQH], BF16)
            qTg_dram = dram.tile([2, D, SQH], BF16)

        # ---- projections (inputs scoped so their SBUF frees afterwards) ----
        with tc.tile_pool(name="wx", bufs=1) as wx:
            wq_sb = wx.tile([P, DT, D], BF16)
            wk_sb = wx.tile([P, DT, D], BF16)
            wv_sb = wx.tile([P, DT, D], BF16)
            xq_sb = wx.tile([P, DT, sq_in], BF16)
            xk_sb = wx.tile([P, DT, SKV], BF16)
            xv_sb = wx.tile([P, DT, SKV], BF16)

            # phase-ordered DMA so each projection's inputs arrive first;
            # the leading phase additionally gets its first tiles early
            for dt in range(DT):
                sl = slice(dt * P, (dt + 1) * P)
                nc.sync.dma_start(out=wq_sb[:, dt, 0:P], in_=wqT_d[sl, 0:P])
                nc.sync.dma_start(out=xq_sb[:, dt, 0:CH], in_=xqT_d[sl, 0:CH])
            for dt in range(DT):
                sl = slice(dt * P, (dt + 1) * P)
                nc.sync.dma_start(out=wq_sb[:, dt, P:D], in_=wqT_d[sl, P:D])
                nc.sync.dma_start(out=xq_sb[:, dt, CH:sq_in], in_=xqT_d[sl, CH:sq_in])
            for dt in range(DT):
                sl = slice(dt * P, (dt + 1) * P)
                nc.sync.dma_start(out=wk_sb[:, dt, :], in_=wkT_d[sl, :])
                nc.sync.dma_start(out=xk_sb[:, dt, :], in_=xkT_d[sl, :])
            for dt in range(DT):
                sl = slice(dt * P, (dt + 1) * P)
                nc.sync.dma_start(out=wv_sb[:, dt, :], in_=wvT_d[sl, :])
                nc.sync.dma_start(out=xv_sb[:, dt, :], in_=xvT_d[sl, :])

            # qT (or qT-half) = Wq @ xq.T (+bq)
            if USE_COLLECTIVE:
                qTh_sb = wx.tile([P, ET, SQH], BF16)
                q_dst = qTh_sb
            else:
                q_dst = qT_sb
            for et in range(ET):
                esl = slice(et * P, (et + 1) * P)
                for qc in range(sq_in // CH):
                    csl = slice(qc * CH, (qc + 1) * CH)
                    ps_q = psum_proj.tile([P, CH], FP32, tag="psproj")
                    for dt in range(DT):
                        nc.tensor.matmul(
                            ps_q,
                            wq_sb[:, dt, esl],
                            xq_sb[:, dt, csl],
                            start=(dt == 0),
                            stop=(dt == DT - 1),
                        )
                    nc.scalar.activation(
                        out=q_dst[:, et, csl],
                        in_=ps_q,
                        func=mybir.ActivationFunctionType.Identity,
                        bias=bq_sb[:, et : et + 1],
                        scale=1.0,
                    )

            if USE_COLLECTIVE:
                # pair-wise AllGather of the q halves via DRAM bounce
                for et in range(ET):
                    esl = slice(et * P, (et + 1) * P)
                    nc.sync.dma_start(out=qTh_dram[esl, :], in_=qTh_sb[:, et, :])
                nc.gpsimd.collective_compute(
                    "AllGather",
                    mybir.AluOpType.bypass,
                    replica_groups=[[0, 1], [2, 3], [4, 5], [6, 7]],
                    ins=[qTh_dram.opt()],
                    outs=[qTg_dram.opt()],
                )
                # read-back on the ACT HWDGE queue: these descriptors wait on
                # the collective, and at the head of the SP queues they would
                # block every later output DMA queued behind them
                for g in range(2):
                    gsl = slice(g * SQH, (g + 1) * SQH)
                    for et in range(ET):
                        esl = slice(et * P, (et + 1) * P)
                        nc.scalar.dma_start(
                            out=qT_sb[:, et, gsl], in_=qTg_dram[g, esl, :]
                        )

            # kT = Wk @ xk.T (+bk)
            for et in range(ET):
                esl = slice(et * P, (et + 1) * P)
                for kc in range(SKV // CH):
                    csl = slice(kc * CH, (kc + 1) * CH)
                    ps_k = psum_proj.tile([P, CH], FP32, tag="psproj")
                    for dt in range(DT):
                        nc.tensor.matmul(
                            ps_k,
                            wk_sb[:, dt, esl],
                            xk_sb[:, dt, csl],
                            start=(dt == 0),
                            stop=(dt == DT - 1),
                        )
                    nc.scalar.activation(
                        out=kT_sb[:, et, csl],
                        in_=ps_k,
                        func=mybir.ActivationFunctionType.Identity,
                        bias=bk_sb[:, et : et + 1],
                        scale=1.0,
                    )

            # v = xv @ Wv.T (no bias)
            for jt in range(JT):
                jsl = slice(jt * P, (jt + 1) * P)
                for ec in range(D // CH):
                    csl = slice(ec * CH, (ec + 1) * CH)
                    ps_v = psum_proj.tile([P, CH], FP32, tag="psproj")
                    for dt in range(DT):
                        nc.tensor.matmul(
                            ps_v,
                            xv_sb[:, dt, jsl],
                            wv_sb[:, dt, csl],
                            start=(dt == 0),
                            stop=(dt == DT - 1),
                        )
                    nc.vector.tensor_copy(v_sb[:, jt, csl], ps_v)

        # ---- attention ----
        for ch in range(NCH):
            csl = slice(ch * CH, (ch + 1) * CH)
            # scoresT[j_tile, chunk] accumulated over e; exp into SBUF bf16
            e_tiles = []
            for jt in range(JT):
                jsl = slice(jt * P, (jt + 1) * P)
                ps_s = psum_s.tile([P, CH], FP32, tag="pss")
                for et in range(ET):
                    nc.tensor.matmul(
                        ps_s,
                        kT_sb[:, et, jsl],
                        qT_sb[:, et, csl],
                        start=(et == 0),
                        stop=(et == ET - 1),
                    )
                e_sb = exp_pool.tile([P, CH], BF16, tag="expt")
                nc.scalar.activation(
                    out=e_sb,
                    in_=ps_s,
                    func=mybir.ActivationFunctionType.Exp,
                    scale=SCALE,
                )
                e_tiles.append(e_sb)

            # sums[1, chunk] = sum_j expT  (ones-matmul, accumulate over j)
            ps_n = psum_n.tile([1, CH], FP32, tag="psn")
            for jt in range(JT):
                nc.tensor.matmul(
                    ps_n,
                    ones_sb[:, :],
                    e_tiles[jt],
                    start=(jt == 0),
                    stop=(jt == JT - 1),
                )
            sums_sb = stage.tile([1, CH], FP32, tag="sums_sb")
            nc.vector.tensor_copy(sums_sb, ps_n)
            nc.sync.dma_start(out=sums_d[:, csl], in_=sums_sb)

            # outT[e_tile, chunk] = sum_j v[j, e_tile].T @ expT[j, chunk]
            for et in range(ET):
                esl = slice(et * P, (et + 1) * P)
                ps_ot = psum_o.tile([P, CH], FP32, tag="pso")
                for jt in range(JT):
                    nc.tensor.matmul(
                        ps_ot,
                        v_sb[:, jt, esl],
                        e_tiles[jt],
                        start=(jt == 0),
                        stop=(jt == JT - 1),
                    )
                o_sb = stage.tile([P, CH], FP32, tag="o_sb")
                nc.vector.tensor_copy(o_sb, ps_ot)
                nc.sync.dma_start(out=outT_d[esl, csl], in_=o_sb)

    # Bacc register allocation / nop fusion / event-sem generation must run
    # before serialization (bass_exec also asserts is_finalized). The wait
    # splitting must run after, so later passes can't re-merge the nops.
    nc.finalize()
    _split_multi_waits(nc)
    return nc


_NC_CACHE = None


def kernel(query, key, value, Wq, bq, Wk, bk, Wv, bv, _trace=False):
    global LAST_EXEC_NS, LAST_RESULT, _NC_CACHE

    query = np.asarray(query, dtype=np.float32)
    key = np.asarray(key, dtype=np.float32)
    value = np.asarray(value, dtype=np.float32)
    Wq = np.asarray(Wq, dtype=np.float32)
    bq = np.asarray(bq, dtype=np.float32)
    Wk = np.asarray(Wk, dtype=np.float32)
    bk = np.asarray(bk, dtype=np.float32)
    Wv = np.asarray(Wv, dtype=np.float32)
    bv = np.asarray(bv, dtype=np.float32)

    bf = ml_dtypes.bfloat16
    wqT = np.ascontiguousarray(Wq.T).astype(bf)
    wkT = np.ascontiguousarray(Wk.T).astype(bf)
    wvT = np.ascontiguousarray(Wv.T).astype(bf)
    bqr = np.ascontiguousarray(bq.reshape(DT, P).T)
    bkr = np.ascontiguousarray(bk.reshape(DT, P).T)

    in_maps = []
    for b in range(B):
        xqT_full = np.ascontiguousarray(query[b].T).astype(bf)  # [D, SQ]
        xkT_full = np.ascontiguousarray(key[b].T).astype(bf)    # [D, 2048]
        xvT_full = np.ascontiguousarray(value[b].T).astype(bf)
        for h in range(2):
            hsl = slice(h * SKV, (h + 1) * SKV)
            if USE_COLLECTIVE:
                xqT = np.ascontiguousarray(xqT_full[:, h * SQH : (h + 1) * SQH])
            else:
                xqT = xqT_full
            in_maps.append(
                {
                    "xqT": xqT,
                    "xkT": np.ascontiguousarray(xkT_full[:, hsl]),
                    "xvT": np.ascontiguousarray(xvT_full[:, hsl]),
                    "wqT": wqT,
                    "wkT": wkT,
                    "wvT": wvT,
                    "bqr": bqr,
                    "bkr": bkr,
                }
            )

    if _NC_CACHE is None:
        _NC_CACHE = _build_bass()
    nc = _NC_CACHE

    res = run_bass_kernel_spmd(
        nc,
        in_maps,
        core_ids=list(range(8)),
        trace=_trace,
    )
    LAST_RESULT = res
    LAST_EXEC_NS = res.exec_time_ns

    out = np.empty((B, SQ, D), dtype=np.float32)
    for b in range(B):
        r0, r1 = res.results[2 * b], res.results[2 * b + 1]
        O = r0["outT"] + r1["outT"]          # [D, SQ]
        s = r0["sums"][0] + r1["sums"][0]    # [SQ]
        out[b] = (O / s[None, :]).T + bv[None, :]
    return out



# revision 3
# speedup vs baseline: 1.0672x; 1.0672x over previous
"""CrossAttention kernel for 8 trn2 NeuronCores.

Sharding: core = (batch b in 0..3, key-half h in 0..1).
Each core computes, for its batch b and its half of the keys:
    qT   = (Wq @ query[b].T + bq)      [E=1024, Sq=2048]
           (computed as a half per core, pair-AllGathered via DRAM bounce)
    kT   = (Wk @ key_half.T + bk)      [E=1024, Skv=1024]
    v    = (value_half @ Wv.T)         [Skv=1024, E=1024]  (bias deferred to host)
    sT   = scoresT[j,i] = k_j . q_i    [Skv, Sq]
    eT   = exp(sT / sqrt(D))           (no max subtraction; scores are O(1))
    outT = outT[d,i] = sum_j v[j,d] eT[j,i]   [E, Sq]  (unnormalized, bf16)
    sums = sum_j eT[j,i]               [1, Sq]  (DVE add-tree + gpsimd
                                        partition_all_reduce — off the PE)
Host combines the two halves per batch:
    out[b] = ((outT0+outT1) / (sums0+sums1)).T + bv
All matmuls run in bf16 with fp32 PSUM accumulation.

Perf notes (vs the first working version):
  - inputs arrive via ~10 large multi-dim DMAs instead of 64 small ones
    (the 0.6us-per-DMA issue cost on the SP queue starved the PE early and
    delayed the collective's DRAM bounce writes by ~25us)
  - the q-half DRAM bounce writes issue per-et on the ACT HWDGE ring,
    interleaved with the q projection, so the AllGather triggers right
    after q-proj instead of ~25us later
  - readback of the gathered q is 4 chunk-granular DMAs (not 16)
  - softmax denominators come off the tensor engine (32 ones-matmuls
    ~7us) onto the idle DVE/GpSimd engines
  - outT is written bf16 (half the output DMA bytes)
"""

from contextlib import ExitStack

import numpy as np
import ml_dtypes

import concourse.bass as bass
import concourse.tile as tile
from concourse import bacc, bass_isa, mybir
from concourse.bass_utils import run_bass_kernel_spmd

BF16 = mybir.dt.bfloat16
FP32 = mybir.dt.float32

B = 4
SQ = 2048        # query length (full batch)
SQH = SQ // 2    # per-core query half
SKV = 1024       # keys per core (half of 2048)
D = 1024         # model dim = proj dim
P = 128          # partitions
CH = 512         # psum free-dim chunk
DT = D // P      # 8 contraction tiles for projections
ET = D // P      # 8 e-tiles
JT = SKV // P    # 8 key tiles per core
NCH = SQ // CH   # 4 sq chunks
SCALE = 1.0 / float(np.sqrt(D))

GPSIMD_SUMS = True

LAST_EXEC_NS = None
LAST_RESULT = None


def _split_multi_waits(nc):
    """The container's walrus supports exactly ONE sync-wait command per
    instruction ("Too many sync wait commands" otherwise). Tile emits
    instructions carrying several waits; split the extras onto same-engine
    NOPs inserted immediately before the instruction (engine streams are
    in-order, so waits still complete before the instruction starts)."""
    ctr = 0
    for fn in nc.m.functions:
        for bb in fn.blocks:
            insts = bb.instructions
            new = []
            changed = False
            for inst in insts:
                si = inst.sync_info
                waits = list(si.on_wait) if si is not None and si.on_wait else []
                if len(waits) > 1:
                    changed = True
                    for w in waits[:-1]:
                        ctr += 1
                        new.append(
                            mybir.InstNoOp(
                                name=f"waitsplit_{ctr}",
                                engine=inst.engine,
                                ins=[],
                                outs=[],
                                sync_info=mybir.SyncInfo(on_wait=[w], on_update=[]),
                            )
                        )
                    inst.sync_info = mybir.SyncInfo(
                        on_wait=[waits[-1]],
                        on_update=list(si.on_update) if si.on_update else [],
                    )
                new.append(inst)
            if changed:
                insts[:] = new
    return ctr


class _SlimTailTileContext(tile.TileContext):
    """Tile's kernel tail is drain + all-engine barrier + semaphore
    range-clear + second barrier. Only the drain (with its global-clock
    waits) is needed for the outputs of THIS execution to be complete when
    every engine halts; the clears/barriers are hygiene for re-executing
    the same loaded NEFF, which we never do."""

    def _drain_and_barrier(self, tick_clock, wait_clock):
        from concourse.vector_clock import ScopedClock

        drain_inst = self.nc.sync.drain()
        wait_clock.add_sem_waits(
            drain_inst.ins, ScopedClock({None: tick_clock.global_clock})
        )
        assert self.sems is not None
        popped = self.nc._tile_sem_poison_stack.pop()
        assert popped is self._sem_poison


def _build_bass():
    nc = bacc.Bacc(
        "TRN2", target_bir_lowering=False, debug=False, num_devices=8
    )

    xqT_d = nc.dram_tensor("xqT", [D, SQH], BF16, kind="ExternalInput")
    xkT_d = nc.dram_tensor("xkT", [D, SKV], BF16, kind="ExternalInput")
    xvT_d = nc.dram_tensor("xvT", [D, SKV], BF16, kind="ExternalInput")
    wqT_d = nc.dram_tensor("wqT", [D, D], BF16, kind="ExternalInput")
    wkT_d = nc.dram_tensor("wkT", [D, D], BF16, kind="ExternalInput")
    wvT_d = nc.dram_tensor("wvT", [D, D], BF16, kind="ExternalInput")
    bqr_d = nc.dram_tensor("bqr", [P, DT], FP32, kind="ExternalInput")
    bkr_d = nc.dram_tensor("bkr", [P, DT], FP32, kind="ExternalInput")
    outT_d = nc.dram_tensor("outT", [D, SQ], BF16, kind="ExternalOutput")
    sums_d = nc.dram_tensor("sums", [1, SQ], FP32, kind="ExternalOutput")

    # [p, dt, c] views of the [D, D]/[D, n] DRAM tensors: partition p of
    # contraction tile dt holds source row dt*128+p.
    wqv = wqT_d.rearrange("(dt p) c -> p dt c", p=P)
    wkv = wkT_d.rearrange("(dt p) c -> p dt c", p=P)
    wvv = wvT_d.rearrange("(dt p) c -> p dt c", p=P)
    xqv = xqT_d.rearrange("(dt p) c -> p dt c", p=P)
    xkv = xkT_d.rearrange("(dt p) c -> p dt c", p=P)
    xvv = xvT_d.rearrange("(dt p) c -> p dt c", p=P)

    with _SlimTailTileContext(nc) as tc, ExitStack() as ctx:
        const_pool = ctx.enter_context(tc.tile_pool(name="const", bufs=1))
        persist = ctx.enter_context(tc.tile_pool(name="persist", bufs=1))
        # attention pools allocated BEFORE the wx scope so their SBUF space
        # does not overlap wx's
        exp_pool = ctx.enter_context(tc.tile_pool(name="expp", bufs=2))
        stage = ctx.enter_context(tc.tile_pool(name="stage", bufs=4))
        sums_pool = ctx.enter_context(tc.tile_pool(name="sumsp", bufs=2))

        bq_sb = const_pool.tile([P, DT], FP32)
        bk_sb = const_pool.tile([P, DT], FP32)
        if not GPSIMD_SUMS:
            ones_sb = const_pool.tile([P, 1], BF16)
            nc.vector.memset(ones_sb, 1.0)

        # persistent projection outputs (bf16)
        qT_sb = persist.tile([P, ET, SQ], BF16)   # [e_in, e_out, sq]
        kT_sb = persist.tile([P, ET, SKV], BF16)  # [e_in, e_out, skv]
        v_sb = persist.tile([P, JT, D], BF16)     # [j_in, j_out, e]

        dram = ctx.enter_context(tc.tile_pool(name="dram", bufs=1, space="DRAM"))
        qTh_dram = dram.tile([D, SQH], BF16)
        qTg_dram = dram.tile([2, D, SQH], BF16)
        qTg_view = qTg_dram.rearrange("g (et p) c -> g p et c", p=P)

        # ---- projections (inputs scoped so their SBUF frees afterwards) ----
        with tc.tile_pool(name="wx", bufs=1) as wx, tc.tile_pool(
            name="psum_proj", bufs=3, space="PSUM"
        ) as psum_proj:
            wq_sb = wx.tile([P, DT, D], BF16)
            wk_sb = wx.tile([P, DT, D], BF16)
            wv_sb = wx.tile([P, DT, D], BF16)
            xq_sb = wx.tile([P, DT, SQH], BF16)
            xk_sb = wx.tile([P, DT, SKV], BF16)
            xv_sb = wx.tile([P, DT, SKV], BF16)
            qTh_sb = wx.tile([P, ET, SQH], BF16)

            # Batched, need-ordered input DMAs on the SP queue. The first
            # psum tile (et=0, qc=0) needs wq cols 0:128 + xq cols 0:512
            # for all dt; split those in dt-halves so matmuls start early.
            nc.sync.dma_start(out=wq_sb[:, 0:4, 0:P], in_=wqv[:, 0:4, 0:P])
            nc.sync.dma_start(out=xq_sb[:, 0:4, 0:CH], in_=xqv[:, 0:4, 0:CH])
            nc.sync.dma_start(out=wq_sb[:, 4:DT, 0:P], in_=wqv[:, 4:DT, 0:P])
            nc.sync.dma_start(out=xq_sb[:, 4:DT, 0:CH], in_=xqv[:, 4:DT, 0:CH])
            nc.sync.dma_start(out=bq_sb, in_=bqr_d[:, :])
            nc.sync.dma_start(out=bk_sb, in_=bkr_d[:, :])
            # et=0 qc=1 needs xq cols 512:1024; et>=1 needs wq cols 128:1024
            nc.sync.dma_start(out=xq_sb[:, :, CH:SQH], in_=xqv[:, :, CH:SQH])
            nc.sync.dma_start(out=wq_sb[:, :, P:D], in_=wqv[:, :, P:D])
            nc.sync.dma_start(out=wk_sb[:, :, :], in_=wkv[:, :, :])
            nc.sync.dma_start(out=xk_sb[:, :, :], in_=xkv[:, :, :])
            nc.sync.dma_start(out=wv_sb[:, :, :], in_=wvv[:, :, :])
            nc.sync.dma_start(out=xv_sb[:, :, :], in_=xvv[:, :, :])

            # qT half = Wq @ xq.T (+bq); bounce each et row to DRAM on the
            # ACT ring as soon as it is complete so the AllGather triggers
            # right after the q projection finishes
            for et in range(ET):
                esl = slice(et * P, (et + 1) * P)
                for qc in range(SQH // CH):
                    csl = slice(qc * CH, (qc + 1) * CH)
                    ps_q = psum_proj.tile([P, CH], FP32, tag="psproj")
                    for dt in range(DT):
                        nc.tensor.matmul(
                            ps_q,
                            wq_sb[:, dt, esl],
                            xq_sb[:, dt, csl],
                            start=(dt == 0),
                            stop=(dt == DT - 1),
                        )
                    nc.scalar.activation(
                        out=qTh_sb[:, et, csl],
                        in_=ps_q,
                        func=mybir.ActivationFunctionType.Identity,
                        bias=bq_sb[:, et : et + 1],
                        scale=1.0,
                    )
                nc.scalar.dma_start(out=qTh_dram[esl, :], in_=qTh_sb[:, et, :])

            # pair-wise AllGather of the q halves via DRAM bounce
            nc.gpsimd.collective_compute(
                "AllGather",
                mybir.AluOpType.bypass,
                replica_groups=[[0, 1], [2, 3], [4, 5], [6, 7]],
                ins=[qTh_dram.opt()],
                outs=[qTg_dram.opt()],
            )
            # chunk-granular readback on the ACT ring (the SP ring is busy
            # streaming inputs); scores for chunk c wait only on DMA c
            for g in range(2):
                for qc in range(SQH // CH):
                    dsl = slice(g * SQH + qc * CH, g * SQH + (qc + 1) * CH)
                    ssl = slice(qc * CH, (qc + 1) * CH)
                    nc.scalar.dma_start(
                        out=qT_sb[:, :, dsl], in_=qTg_view[g, :, :, ssl]
                    )

            # kT = Wk @ xk.T (+bk)
            for et in range(ET):
                esl = slice(et * P, (et + 1) * P)
                for kc in range(SKV // CH):
                    csl = slice(kc * CH, (kc + 1) * CH)
                    ps_k = psum_proj.tile([P, CH], FP32, tag="psproj")
                    for dt in range(DT):
                        nc.tensor.matmul(
                            ps_k,
                            wk_sb[:, dt, esl],
                            xk_sb[:, dt, csl],
                            start=(dt == 0),
                            stop=(dt == DT - 1),
                        )
                    nc.scalar.activation(
                        out=kT_sb[:, et, csl],
                        in_=ps_k,
                        func=mybir.ActivationFunctionType.Identity,
                        bias=bk_sb[:, et : et + 1],
                        scale=1.0,
                    )

            # v = xv @ Wv.T (no bias)
            for jt in range(JT):
                jsl = slice(jt * P, (jt + 1) * P)
                for ec in range(D // CH):
                    csl = slice(ec * CH, (ec + 1) * CH)
                    ps_v = psum_proj.tile([P, CH], FP32, tag="psproj")
                    for dt in range(DT):
                        nc.tensor.matmul(
                            ps_v,
                            xv_sb[:, dt, jsl],
                            wv_sb[:, dt, csl],
                            start=(dt == 0),
                            stop=(dt == DT - 1),
                        )
                    nc.vector.tensor_copy(v_sb[:, jt, csl], ps_v)

        psum_s = ctx.enter_context(tc.tile_pool(name="psum_s", bufs=4, space="PSUM"))
        psum_o = ctx.enter_context(tc.tile_pool(name="psum_o", bufs=4, space="PSUM"))
        if not GPSIMD_SUMS:
            psum_n = ctx.enter_context(
                tc.tile_pool(name="psum_n", bufs=1, space="PSUM")
            )

        # ---- attention ----
        for ch in range(NCH):
            csl = slice(ch * CH, (ch + 1) * CH)
            # scoresT[j_tile, chunk] accumulated over e; exp into SBUF bf16
            e_big = exp_pool.tile([P, JT, CH], BF16, tag="expt")
            for jt in range(JT):
                jsl = slice(jt * P, (jt + 1) * P)
                ps_s = psum_s.tile([P, CH], FP32, tag="pss")
                for et in range(ET):
                    nc.tensor.matmul(
                        ps_s,
                        kT_sb[:, et, jsl],
                        qT_sb[:, et, csl],
                        start=(et == 0),
                        stop=(et == ET - 1),
                    )
                nc.scalar.activation(
                    out=e_big[:, jt, :],
                    in_=ps_s,
                    func=mybir.ActivationFunctionType.Exp,
                    scale=SCALE,
                )

            # sums[1, chunk] = sum_j expT — off the PE: DVE add-tree over
            # the 8 j-tiles, then a gpsimd cross-partition reduce
            if GPSIMD_SUMS:
                acc = sums_pool.tile([P, CH], FP32, tag="sacc")
                nc.vector.tensor_add(acc, e_big[:, 0, :], e_big[:, 1, :])
                for jt in range(2, JT):
                    nc.vector.tensor_add(acc, acc, e_big[:, jt, :])
                red = sums_pool.tile([P, CH], FP32, tag="sred")
                nc.gpsimd.partition_all_reduce(
                    red, acc, P, bass_isa.ReduceOp.add
                )
                nc.sync.dma_start(out=sums_d[:, csl], in_=red[0:1, :])
            else:
                ps_n = psum_n.tile([1, CH], FP32, tag="psn")
                for jt in range(JT):
                    nc.tensor.matmul(
                        ps_n,
                        ones_sb[:, :],
                        e_big[:, jt, :],
                        start=(jt == 0),
                        stop=(jt == JT - 1),
                    )
                sums_sb = sums_pool.tile([1, CH], FP32, tag="sums_sb")
                nc.vector.tensor_copy(sums_sb, ps_n)
                nc.sync.dma_start(out=sums_d[:, csl], in_=sums_sb)

            # outT[e_tile, chunk] = sum_j v[j, e_tile].T @ expT[j, chunk]
            for et in range(ET):
                esl = slice(et * P, (et + 1) * P)
                ps_ot = psum_o.tile([P, CH], FP32, tag="pso")
                for jt in range(JT):
                    nc.tensor.matmul(
                        ps_ot,
                        v_sb[:, jt, esl],
                        e_big[:, jt, :],
                        start=(jt == 0),
                        stop=(jt == JT - 1),
                    )
                o_sb = stage.tile([P, CH], BF16, tag="o_sb")
                nc.vector.tensor_copy(o_sb, ps_ot)
                nc.sync.dma_start(out=outT_d[esl, csl], in_=o_sb)

    # Bacc register allocation / nop fusion / event-sem generation must run
    # before serialization (bass_exec also asserts is_finalized). The wait
    # splitting must run after, so later passes can't re-merge the nops.
    nc.finalize()
    _split_multi_waits(nc)
    return nc


_NC_CACHE = None


def kernel(query, key, value, Wq, bq, Wk, bk, Wv, bv, _trace=False):
    global LAST_EXEC_NS, LAST_RESULT, _NC_CACHE

    query = np.asarray(query, dtype=np.float32)
    key = np.asarray(key, dtype=np.float32)
    value = np.asarray(value, dtype=np.float32)
    Wq = np.asarray(Wq, dtype=np.float32)
    bq = np.asarray(bq, dtype=np.float32)
    Wk = np.asarray(Wk, dtype=np.float32)
    bk = np.asarray(bk, dtype=np.float32)
    Wv = np.asarray(Wv, dtype=np.float32)
    bv = np.asarray(bv, dtype=np.float32)

    bf = ml_dtypes.bfloat16
    wqT = np.ascontiguousarray(Wq.T).astype(bf)
    wkT = np.ascontiguousarray(Wk.T).astype(bf)
    wvT = np.ascontiguousarray(Wv.T).astype(bf)
    bqr = np.ascontiguousarray(bq.reshape(DT, P).T)
    bkr = np.ascontiguousarray(bk.reshape(DT, P).T)

    in_maps = []
    for b in range(B):
        xqT_full = np.ascontiguousarray(query[b].T).astype(bf)  # [D, SQ]
        xkT_full = np.ascontiguousarray(key[b].T).astype(bf)    # [D, 2048]
        xvT_full = np.ascontiguousarray(value[b].T).astype(bf)
        for h in range(2):
            hsl = slice(h * SKV, (h + 1) * SKV)
            xqT = np.ascontiguousarray(xqT_full[:, h * SQH : (h + 1) * SQH])
            in_maps.append(
                {
                    "xqT": xqT,
                    "xkT": np.ascontiguousarray(xkT_full[:, hsl]),
                    "xvT": np.ascontiguousarray(xvT_full[:, hsl]),
                    "wqT": wqT,
                    "wkT": wkT,
                    "wvT": wvT,
                    "bqr": bqr,
                    "bkr": bkr,
                }
            )

    if _NC_CACHE is None:
        _NC_CACHE = _build_bass()
    nc = _NC_CACHE

    res = run_bass_kernel_spmd(
        nc,
        in_maps,
        core_ids=list(range(8)),
        trace=_trace,
    )
    LAST_RESULT = res
    LAST_EXEC_NS = res.exec_time_ns

    out = np.empty((B, SQ, D), dtype=np.float32)
    for b in range(B):
        r0, r1 = res.results[2 * b], res.results[2 * b + 1]
        O = r0["outT"].astype(np.float32) + r1["outT"].astype(np.float32)
        s = r0["sums"][0] + r1["sums"][0]    # [SQ]
        out[b] = (O / s[None, :]).T + bv[None, :]
    return out
